# revision 24
# baseline (speedup 1.0000x reference)
"""Multi-head self-attention with RoPE on 8 Trainium2 NeuronCores.

Sharding: 2-D (batch x head-half). Core c owns batch c//2 and heads
8*(c%2)..8*(c%2)+7 - a 512-wide slice of the Wq/Wk/Wv output dims and the
matching Wo rows, over 2048 tokens. Each core computes q/k/v for its head
slice, runs causal RoPE attention, and produces a partial output projection
yT_c = Wo_slice^T @ O_c. The host sums core pairs (the TP all-reduce) and
transposes back.

Layout is feature-major (dims on partitions, tokens on the free axis), all
matmul operands bf16 (1 cyc/row at any moving size, half the DMA/SBUF of
f32r). Scores/projection accumulate in f32 PSUM. RoPE uses a host-side
[evens|odds] permutation of the wq/wk output columns so the rotate step is
partition-block multiplies; tables are premultiplied with the +-sin sign.
Softmax skips max-subtraction (scores ~N(0,1) by construction); the
denominator comes from a ones-column appended per head in V and is divided
out after the AV matmul via a gpsimd partition-broadcast of 1/denom.

Scheduling targets the PE HAM clock gate (PE idles re-throttle it to
1.2 GHz): attention score matmuls for head h+1 are emitted before the AV
burst of head h, exp instructions are paired over two 512-col chunks to
amortize Activation-engine access latency, causal-diagonal chunks compute
only the surviving columns, and projection/output-projection matmul groups
are woven between attention chunks as always-ready PE filler.
"""

from collections import deque

import numpy as np
import ml_dtypes

# Problem shapes (fixed by the task).
B, N, D = 4, 2048, 1024
H, DK = 16, 64
THETA = 10000.0
NCORES = 8
HPC = 8                    # heads per core
DS = HPC * DK              # head-dim slice per core = 512
P = 128                    # SBUF partitions
QT = 512                   # q-token tile
KC = 128                   # k-token chunk (contraction partition dim)
NQT = N // QT              # 4
NKC = N // KC              # 16
MASK_NEG = -1.0e30
NPB = N // P               # 16 v row-chunks

BF16NP = ml_dtypes.bfloat16
VW = 128                   # v columns per head: 64 values + 64 ones columns
                           # (the AV matmul then emits the softmax denominator
                           # replicated on partitions 64..127 - no broadcast)


def _split_drain_tile_context(tile_mod, bass_rust_mod, vector_clock_mod):
    """TileContext subclass that splits the tail drain's semaphore waits
    into one drain instruction per wait - this walrus build rejects CTRL
    instructions carrying more than one sync wait."""

    class TileContextSplitDrain(tile_mod.TileContext):
        def _drain_and_barrier(self, tick_clock, wait_clock):
            drain_inst = self.nc.sync.drain()
            wait_clock.add_sem_waits(
                drain_inst.ins,
                vector_clock_mod.ScopedClock({None: tick_clock.global_clock}),
            )
            si = drain_inst.ins.sync_info
            waits = list(si.on_wait) if si is not None else []
            if len(waits) > 1:
                drain_inst.ins.sync_info = bass_rust_mod.SyncInfo(
                    on_wait=[waits[0]], on_update=list(si.on_update)
                )
                for w in waits[1:]:
                    extra = self.nc.sync.drain()
                    extra.ins.sync_info = bass_rust_mod.SyncInfo(
                        on_wait=[w], on_update=[]
                    )
            self.nc.all_engine_barrier()
            assert self.sems is not None
            popped = self.nc._tile_sem_poison_stack.pop()
            assert popped is self._sem_poison
            self.nc.clear_and_free_semaphores(list(self.sems.allocated().values()))
            self.nc.all_engine_barrier()

    return TileContextSplitDrain


def _split_excess_waits(nc, mybir, max_waits=1):
    """This walrus build rejects instructions carrying more than one sync
    wait ("Too many sync wait commands"). Move excess waits onto preceding
    same-engine NOPs, which enforce them sequentially."""
    nid = 0
    for fn in nc.m.functions:
        for bb in fn.blocks:
            new = []
            changed = False
            for inst in bb.instructions:
                si = inst.sync_info
                waits = list(si.on_wait) if si is not None else []
                if len(waits) > max_waits:
                    changed = True
                    excess, keep = waits[:-max_waits], waits[-max_waits:]
                    for w in excess:
                        nid += 1
                        nop = mybir.InstNoOp(
                            name=f"I-waitsplit-{nid}-{inst.name}",
                            sync_info=mybir.SyncInfo(on_wait=[w], on_update=[]),
                            engine=inst.engine,
                            bass_nofuse=True,
                        )
                        nc.register_instruction(nop, overwrite=True)
                        new.append(nop)
                    inst.sync_info = mybir.SyncInfo(
                        on_wait=keep, on_update=list(si.on_update))
                new.append(inst)
            if changed:
                bb.instructions = new


def build_mhsa():
    """Build the SPMD Bass program (identical on all cores; per-core inputs
    carry each core's batch slice and weight slices)."""
    from contextlib import ExitStack

    import bass_rust
    import concourse.bass as bass
    import concourse.mybir as mybir
    import concourse.tile as tile
    import concourse.vector_clock as vector_clock

    TC = _split_drain_tile_context(tile, bass_rust, vector_clock)
    f32 = mybir.dt.float32
    bf16 = mybir.dt.bfloat16
    Act = bass_rust.ActivationFunctionType

    nc = bass.Bass("TRN2", target_bir_lowering=False, debug=False,
                   num_devices=NCORES)
    xT = nc.dram_tensor("xT", [D, N], bf16, kind="ExternalInput").ap()
    wq = nc.dram_tensor("wq", [D, DS], bf16, kind="ExternalInput").ap()
    wk = nc.dram_tensor("wk", [D, DS], bf16, kind="ExternalInput").ap()
    wv = nc.dram_tensor("wv", [D, DS], bf16, kind="ExternalInput").ap()
    woT = nc.dram_tensor("woT", [DS, D], bf16, kind="ExternalInput").ap()
    ropec = nc.dram_tensor("ropec", [P, N], bf16, kind="ExternalInput").ap()
    ropes = nc.dram_tensor("ropes", [P, N], bf16, kind="ExternalInput").ap()
    tri = nc.dram_tensor("tri", [P, KC], bf16, kind="ExternalInput").ap()
    yT = nc.dram_tensor("yT", [D, N], bf16, kind="ExternalOutput").ap()

    with TC(nc) as tc, ExitStack() as ctx:
        pool = lambda name, bufs, **kw: ctx.enter_context(
            tc.tile_pool(name=name, bufs=bufs, **kw))

        consts = pool("consts", 1)
        # DMA order tracks first use: wq + the t=0 x tiles feed the first
        # matmuls, then wk/wv, then tables (Act DGE queue), woT last
        w_sb = {}
        for nm in ("wq", "wk", "wv"):
            w_sb[nm] = consts.tile([P, D // P, DS], bf16, tag=f"w_{nm}",
                                   name=f"w_{nm}")
        wq_r = wq.rearrange("(c p) m -> p c m", p=P)
        ct_sb = consts.tile([P, N], bf16)
        st_sb = consts.tile([P, N], bf16)
        tri_sb = consts.tile([P, KC], bf16)
        woT_sb = consts.tile([P, DS // P, D], bf16)

        persist = pool("persist", 1)
        qT = persist.tile([P, 4, N], bf16, tag="qT")
        kT = persist.tile([P, 4, N], bf16, tag="kT")
        OT = persist.tile([P, 4, N], bf16, tag="OT")
        vsb = persist.tile([P, NKC, HPC * VW], bf16, tag="vsb")

        xt_pool = pool("xt", 12)
        cp_pool = pool("cp", 3)
        rp_pool = pool("rp", 6)
        es_pool = pool("es", 12)
        dn_pool = pool("dn", 2)
        oc_pool = pool("oc", 3)

        pp_pool = pool("pp", 1, space="PSUM")  # tags ps(3x2) + po(2x1) banks

        def ps_tile():
            return pp_pool.tile([P, 2, QT], f32, tag="ps", bufs=3, name="ps")

        # ones columns in V (softmax denominators), set once
        ones = bass.AP(tensor=vsb.tensor, offset=vsb.offset + 64,
                       ap=[vsb.ap[0], [HPC * VW, NKC], [VW, HPC], [1, 64]])
        nc.vector.memset(ones, 1.0)

        xts = {}

        def emit_xts_dma(t):
            tiles = []
            for c in range(D // P):
                xt = xt_pool.tile([P, QT], bf16, tag="xt", name=f"xt{t}_{c}")
                eng = nc.sync if c < 4 else nc.scalar
                eng.dma_start(
                    out=xt, in_=xT[c * P:(c + 1) * P, t * QT:(t + 1) * QT])
                tiles.append(xt)
            xts[t] = tiles

        def emit_qk_half(t, which, dc, pq, i):
            """8 accumulating matmuls: projection of dim-chunk dc into
            pq[:, i, :]."""
            for c in range(D // P):
                nc.tensor.matmul(pq[:, i, :],
                                 w_sb[which][:, c, dc * P:(dc + 1) * P],
                                 xts[t][c],
                                 start=(c == 0), stop=(c == D // P - 1))

        def emit_qk_rope(t, which, pair, pq):
            """Evict the psum pair through RoPE into qT/kT. The prologue
            (t=0) has an idle Act engine, so it takes the psum eviction
            there; later tiles keep it on DVE (Act is exp-bound then)."""
            tcols = slice(t * QT, (t + 1) * QT)
            cp = cp_pool.tile([P, 2, QT], bf16, tag="cp", name="cp")
            if t == 0:
                nc.scalar.copy(cp, pq)
            else:
                nc.vector.tensor_copy(cp, pq)
            dst = qT if which == "wq" else kT
            for i in (0, 1):
                dc = pair * 2 + i
                # DVE requires equal base partitions for two SBUF inputs, so
                # the sin table is stored block-swapped on the host: st[src]
                # holds the coefficient for dst = src +- 32. Output base may
                # differ (32-part ops route cross-quadrant via bank 0).
                swp = rp_pool.tile([P, QT], bf16, tag="swp", name="swp")
                for g in (0, 1):
                    o = g * 64
                    nc.vector.tensor_mul(swp[o:o + 32, :],
                                         st_sb[o + 32:o + 64, tcols],
                                         cp[o + 32:o + 64, i, :])
                    nc.vector.tensor_mul(swp[o + 32:o + 64, :],
                                         st_sb[o:o + 32, tcols],
                                         cp[o:o + 32, i, :])
                csn = rp_pool.tile([P, QT], bf16, tag="csn", name="csn")
                nc.vector.tensor_mul(csn, ct_sb[:, tcols], cp[:, i, :])
                nc.vector.tensor_add(dst[:, dc, tcols], csn, swp)

        def emit_v_half(t, tk, pv, i):
            for c in range(D // P):
                nc.tensor.matmul(pv[:, i, :],
                                 xts[t][c][:, tk * P:(tk + 1) * P],
                                 w_sb["wv"][:, c, :],
                                 start=(c == 0), stop=(c == D // P - 1))

        def emit_v_evict(t, pair, pv):
            ci0 = t * 4 + pair * 2
            src = bass.AP(tensor=pv.tensor, offset=pv.offset,
                          ap=[pv.ap[0], [pv.ap[1][0], 2], [64, HPC], [1, 64]])
            dstv = bass.AP(tensor=vsb.tensor,
                           offset=vsb.offset + ci0 * (HPC * VW),
                           ap=[vsb.ap[0], [HPC * VW, 2], [VW, HPC], [1, 64]])
            if t == 0:
                nc.scalar.copy(dstv, src)
            else:
                nc.vector.tensor_copy(dstv, src)

        # ---- filler units: always-ready PE work woven into attention ----
        filler = deque()

        def make_a_units(t):
            units = [lambda t=t: emit_xts_dma(t)]
            for pair in (0, 1):
                for which in ("wq", "wk"):
                    state = {}

                    def u1(t=t, which=which, pair=pair, state=state):
                        pq = ps_tile()
                        state["pq"] = pq
                        emit_qk_half(t, which, pair * 2, pq, 0)

                    def u2(t=t, which=which, pair=pair, state=state):
                        pq = state["pq"]
                        emit_qk_half(t, which, pair * 2 + 1, pq, 1)
                        emit_qk_rope(t, which, pair, pq)

                    units += [u1, u2]
            for pair in (0, 1):
                state = {}

                def v1(t=t, pair=pair, state=state):
                    pv = ps_tile()
                    state["pv"] = pv
                    emit_v_half(t, pair * 2, pv, 0)

                def v2(t=t, pair=pair, state=state):
                    pv = state["pv"]
                    emit_v_half(t, pair * 2 + 1, pv, 1)
                    emit_v_evict(t, pair, pv)

                units += [v1, v2]
            return units

        def make_c_units(qt):
            qcols = slice(qt * QT, (qt + 1) * QT)
            units = []
            for j in range(4):
                def cu(qt=qt, j=j, qcols=qcols):
                    pc = ps_tile()
                    for i in (0, 1):
                        m8 = j * 2 + i
                        for dcc in range(DS // P):
                            nc.tensor.matmul(
                                pc[:, i, :],
                                woT_sb[:, dcc, m8 * P:(m8 + 1) * P],
                                OT[:, dcc, qcols],
                                start=(dcc == 0), stop=(dcc == DS // P - 1))
                    oc = oc_pool.tile([P, 2, QT], bf16, tag="oc", name="oc")
                    nc.vector.tensor_copy(oc, pc)
                    for i in (0, 1):
                        m8 = j * 2 + i
                        nc.sync.dma_start(
                            out=yT[m8 * P:(m8 + 1) * P, qcols],
                            in_=oc[:, i, :])
                units.append(cu)
            return units

        def pop_filler(k=1):
            for _ in range(k):
                if filler:
                    filler.popleft()()

        # ---- attention ----
        def emit_scores(qt, h):
            """S + mask + exp stream for one head; returns the AV plan."""
            p0 = (h % 2) * 64
            dc = h // 2
            nkq = 4 * qt + 4
            av = []
            # diagonal chunks first (their masks/exp clear DVE/Act early),
            # merged in pairs: one exp covers both slices from the lower
            # chunk's column offset. The earlier columns of the upper slice
            # exp stale psum (finite scores/projections), and AV never reads
            # them.
            for mp in range(2):
                ps = ps_tile()
                base = mp * 2 * KC
                e = es_pool.tile([P, 2, QT], bf16, tag="es", name="es")
                for i in (0, 1):
                    m = mp * 2 + i
                    kc = 4 * qt + m
                    lo = m * KC
                    nc.tensor.matmul(
                        ps[:, i, lo:QT],
                        kT[p0:p0 + DK, dc, kc * KC:(kc + 1) * KC],
                        qT[p0:p0 + DK, dc, qt * QT + lo:(qt + 1) * QT],
                        start=True, stop=True)
                    av.append((kc, e, i, lo))
                nc.scalar.activation(e[:, :, base:QT], ps[:, :, base:QT],
                                     Act.Exp)
                # multiplicative causal mask on the boundary blocks: zeroes
                # masked entries in e before the denominator-summing AV (all
                # bf16 SBUF, 2x DVE; also keeps exp off the DVE dep chain)
                for i in (0, 1):
                    lo = (mp * 2 + i) * KC
                    nc.vector.tensor_mul(e[:, i, lo:lo + KC],
                                         e[:, i, lo:lo + KC], tri_sb)
            # off-diagonal pairs
            for pr in range(2 * qt):
                ps = ps_tile()
                for i in (0, 1):
                    kc = pr * 2 + i
                    nc.tensor.matmul(
                        ps[:, i, :],
                        kT[p0:p0 + DK, dc, kc * KC:(kc + 1) * KC],
                        qT[p0:p0 + DK, dc, qt * QT:(qt + 1) * QT],
                        start=True, stop=True)
                e = es_pool.tile([P, 2, QT], bf16, tag="es", name="es")
                nc.scalar.activation(e, ps, Act.Exp)
                av.append((pr * 2, e, 0, 0))
                av.append((pr * 2 + 1, e, 1, 0))
                if pr % 2 == 1:
                    pop_filler()
            av.sort(key=lambda z: z[0])
            return (qt, h, nkq, av)

        def emit_av(plan):
            """AV burst + denominator for one head."""
            qt, h, nkq, av = plan
            p0 = (h % 2) * 64
            dc = h // 2
            qcols = slice(qt * QT, (qt + 1) * QT)
            po = pp_pool.tile([P, QT], f32, tag="po", bufs=2, name="po")
            for kc, e, i, lo in av:
                nc.tensor.matmul(
                    po[:, lo:QT],
                    vsb[:, kc, h * VW:(h + 1) * VW],
                    e[:, i, lo:QT],
                    start=(kc == 0), stop=(kc == nkq - 1))
            # 1/d = exp(-ln d): Ln and Exp are co-resident in one Act
            # table set, so no table thrash; DVE reciprocal is the iterative
            # divide (~3.4us per 512 cols) and custom-DVE ops fail codegen.
            lnt = dn_pool.tile([64, QT], f32, tag="lnt", name="lnt")
            nc.scalar.activation(lnt, po[64:128, :], Act.Ln)
            rc = dn_pool.tile([64, QT], f32, tag="rc", name="rc")
            nc.scalar.activation(rc, lnt, Act.Exp, scale=-1.0)
            nc.vector.tensor_mul(OT[p0:p0 + DK, dc, qcols], po[0:64, :], rc)

        # ---- schedule ----
        # A(0) fully inline, Q/K before V: the V matmuls keep the PE busy
        # while DVE finishes the t=0 rope (AV(0,h0) needs all of V(0), so V
        # cannot trail into the filler queue - FIFO head-block). C(qt)
        # fillers are deferred to late q-tiles, which are exp-bound and need
        # the most always-ready PE work.
        tiles0 = []
        for c in range(D // P):
            eng = nc.sync if c < 4 else nc.scalar
            eng.dma_start(out=w_sb["wq"][:, c, :], in_=wq_r[:, c, :])
            xt = xt_pool.tile([P, QT], bf16, tag="xt", name=f"xt0_{c}")
            eng.dma_start(
                out=xt, in_=xT[c * P:(c + 1) * P, 0:QT])
            tiles0.append(xt)
        xts[0] = tiles0
        nc.sync.dma_start(out=w_sb["wk"],
                          in_=wk.rearrange("(c p) m -> p c m", p=P))
        nc.sync.dma_start(out=w_sb["wv"],
                          in_=wv.rearrange("(c p) m -> p c m", p=P))
        nc.scalar.dma_start(out=ct_sb, in_=ropec)
        nc.scalar.dma_start(out=st_sb, in_=ropes)
        nc.scalar.dma_start(out=tri_sb, in_=tri)
        nc.scalar.dma_start(out=woT_sb, in_=woT.rearrange("(c p) m -> p c m", p=P))
        for u in make_a_units(0)[1:]:
            u()
        a1_units = make_a_units(1)
        a1_units[0]()   # xts(1) DMA
        a1_units[1]()   # first Q half-group (8 MMs, no rope): PE work while
                        # the t=0 rope (DVE) finishes before B(0) scores

        pending = None
        for qt in range(NQT):
            if pending is not None:
                # flush before C fillers enter the queue: their matmuls
                # transitively depend on this AV burst (FIFO head-block)
                emit_av(pending)
                pending = None
            if qt == 0:
                filler.extend(a1_units[2:])
            elif qt + 1 < NQT:
                filler.extend(make_a_units(qt + 1))
            if qt == 3:
                for cj in range(3):
                    filler.extend(make_c_units(cj))
            for h in range(HPC):
                plan = emit_scores(qt, h)
                if pending is not None:
                    emit_av(pending)
                pending = plan
                pop_filler()
            # drain leftover fillers between q-tiles
            pop_filler(len(filler))
        emit_av(pending)
        for u in make_c_units(NQT - 1):
            u()
    _split_excess_waits(nc, mybir)
    return nc


def _rope_perm():
    """Per-head output-dim permutation: [evens, odds] per 64-dim head."""
    perm = []
    for h in range(HPC):
        perm += [h * DK + 2 * i for i in range(DK // 2)]
        perm += [h * DK + 2 * i + 1 for i in range(DK // 2)]
    return np.asarray(perm)


def host_inputs(x, Wq, Wk, Wv, Wo):
    """Shard + lay out inputs for each core. Returns list of in_maps."""
    x = np.asarray(x, np.float32)

    perm = _rope_perm()  # over 512 dims (8 heads)
    # RoPE tables in the permuted [evens, odds] partition layout; the
    # 128-partition tile covers 2 heads (identical pattern per 64 block).
    j = np.arange(DK // 2, dtype=np.float32)
    freqs = 1.0 / THETA ** (2.0 * j / DK)               # (32,)
    pos = np.arange(N, dtype=np.float32)
    ang = pos[None, :] * freqs[:, None]                 # (32, n)
    cos_t, sin_t = np.cos(ang), np.sin(ang)
    ct = np.empty((P, N), np.float32)
    st = np.empty((P, N), np.float32)
    # st is stored block-swapped: row p holds the coefficient applied while
    # READING partition p, whose product lands at partition p -+ 32 (see
    # emit_qk_rope). Evens-slot output (p-32) needs -sin; odds-slot (p+32)
    # needs +sin.
    for g in range(2):
        o = g * DK
        ct[o:o + 32] = cos_t
        ct[o + 32:o + 64] = cos_t
        st[o:o + 32] = sin_t
        st[o + 32:o + 64] = -sin_t

    # Triangular multiplicative 0/1 mask for a diagonal boundary block.
    i = np.arange(KC)[:, None]
    jj = np.arange(KC)[None, :]
    tri = np.where(jj >= i, 1.0, 0.0).astype(BF16NP)

    bf = lambda a: np.ascontiguousarray(a).astype(BF16NP)
    scale = 1.0 / np.sqrt(np.float32(DK))
    in_maps = []
    for c in range(NCORES):
        b, a = divmod(c, 2)
        sl = slice(a * DS, (a + 1) * DS)
        xTb = x[b].reshape(N, D).T                       # (D, N)
        wq_c = (np.asarray(Wq, np.float32)[sl, :][perm, :] * scale).T
        wk_c = np.asarray(Wk, np.float32)[sl, :][perm, :].T
        wv_c = np.asarray(Wv, np.float32)[sl, :].T
        woT_c = np.asarray(Wo, np.float32)[:, sl].T
        in_maps.append({
            "xT": bf(xTb),
            "wq": bf(wq_c),
            "wk": bf(wk_c),
            "wv": bf(wv_c),
            "woT": bf(woT_c),
            "ropec": bf(ct),
            "ropes": bf(st),
            "tri": tri,
        })
    return in_maps


def host_gather(results):
    """Sum per-core partial yT outputs and restore (b, n, D) layout."""
    y = np.empty((B, N, D), np.float32)
    for b in range(B):
        acc = results[2 * b]["yT"].astype(np.float32) \
            + results[2 * b + 1]["yT"].astype(np.float32)
        y[b] = acc.T
    return y


def kernel(x, Wq, Wk, Wv, Wo):
    from concourse.bass_utils import run_bass_kernel_spmd

    nc = build_mhsa()
    in_maps = host_inputs(x, Wq, Wk, Wv, Wo)
    res = run_bass_kernel_spmd(nc, in_maps, list(range(NCORES)))
    return host_gather(res.results)


if __name__ == "__main__":
    rng = np.random.default_rng(0)
    x = rng.standard_normal((B, N, D), dtype=np.float32)
    std = (2.0 / (D + D)) ** 0.5
    ws = [rng.standard_normal((D, D), dtype=np.float32) * std for _ in range(4)]
    y = kernel(x, *ws)
    print("kernel ran, output", y.shape, y.dtype)


# revision 26
# speedup vs baseline: 1.0417x; 1.0417x over previous
"""Multi-head self-attention with RoPE on 8 Trainium2 NeuronCores.

Sharding: 2-D (batch x head-half). Core c owns batch c//2 and heads
8*(c%2)..8*(c%2)+7 - a 512-wide slice of the Wq/Wk/Wv output dims and the
matching Wo rows, over 2048 tokens. Each core computes q/k/v for its head
slice, runs causal RoPE attention, and produces a partial output projection
yT_c = Wo_slice^T @ O_c. The host sums core pairs (the TP all-reduce) and
transposes back.

Layout is feature-major (dims on partitions, tokens on the free axis), all
matmul operands bf16 (1 cyc/row at any moving size, half the DMA/SBUF of
f32r). Scores/projection accumulate in f32 PSUM. RoPE uses a host-side
[evens|odds] permutation of the wq/wk output columns so the rotate step is
partition-block multiplies; tables are premultiplied with the +-sin sign.
Softmax skips max-subtraction (scores ~N(0,1) by construction); the
denominator comes from a ones-column appended per head in V and is divided
out after the AV matmul via a gpsimd partition-broadcast of 1/denom.

Scheduling targets the PE HAM clock gate (PE idles re-throttle it to
1.2 GHz): attention score matmuls for head h+1 are emitted before the AV
burst of head h, exp instructions are paired over two 512-col chunks to
amortize Activation-engine access latency, causal-diagonal chunks compute
only the surviving columns, and projection/output-projection matmul groups
are woven between attention chunks as always-ready PE filler.
"""

from collections import deque

import numpy as np
import ml_dtypes

# Problem shapes (fixed by the task).
B, N, D = 4, 2048, 1024
H, DK = 16, 64
THETA = 10000.0
NCORES = 8
HPC = 8                    # heads per core
DS = HPC * DK              # head-dim slice per core = 512
P = 128                    # SBUF partitions
QT = 512                   # q-token tile
KC = 128                   # k-token chunk (contraction partition dim)
NQT = N // QT              # 4
NKC = N // KC              # 16
MASK_NEG = -1.0e30
NPB = N // P               # 16 v row-chunks

BF16NP = ml_dtypes.bfloat16
VW = 128                   # v columns per head: 64 values + 64 ones columns
                           # (the AV matmul then emits the softmax denominator
                           # replicated on partitions 64..127 - no broadcast)


def _split_drain_tile_context(tile_mod, bass_rust_mod, vector_clock_mod):
    """TileContext subclass that splits the tail drain's semaphore waits
    into one drain instruction per wait - this walrus build rejects CTRL
    instructions carrying more than one sync wait."""

    class TileContextSplitDrain(tile_mod.TileContext):
        def _drain_and_barrier(self, tick_clock, wait_clock):
            drain_inst = self.nc.sync.drain()
            wait_clock.add_sem_waits(
                drain_inst.ins,
                vector_clock_mod.ScopedClock({None: tick_clock.global_clock}),
            )
            si = drain_inst.ins.sync_info
            waits = list(si.on_wait) if si is not None else []
            if len(waits) > 1:
                drain_inst.ins.sync_info = bass_rust_mod.SyncInfo(
                    on_wait=[waits[0]], on_update=list(si.on_update)
                )
                for w in waits[1:]:
                    extra = self.nc.sync.drain()
                    extra.ins.sync_info = bass_rust_mod.SyncInfo(
                        on_wait=[w], on_update=[]
                    )
            self.nc.all_engine_barrier()
            assert self.sems is not None
            popped = self.nc._tile_sem_poison_stack.pop()
            assert popped is self._sem_poison
            self.nc.clear_and_free_semaphores(list(self.sems.allocated().values()))
            self.nc.all_engine_barrier()

    return TileContextSplitDrain


def _split_excess_waits(nc, mybir, max_waits=1):
    """This walrus build rejects instructions carrying more than one sync
    wait ("Too many sync wait commands"). Move excess waits onto preceding
    same-engine NOPs, which enforce them sequentially."""
    nid = 0
    for fn in nc.m.functions:
        for bb in fn.blocks:
            new = []
            changed = False
            for inst in bb.instructions:
                si = inst.sync_info
                waits = list(si.on_wait) if si is not None else []
                if len(waits) > max_waits:
                    changed = True
                    excess, keep = waits[:-max_waits], waits[-max_waits:]
                    for w in excess:
                        nid += 1
                        nop = mybir.InstNoOp(
                            name=f"I-waitsplit-{nid}-{inst.name}",
                            sync_info=mybir.SyncInfo(on_wait=[w], on_update=[]),
                            engine=inst.engine,
                            bass_nofuse=True,
                        )
                        nc.register_instruction(nop, overwrite=True)
                        new.append(nop)
                    inst.sync_info = mybir.SyncInfo(
                        on_wait=keep, on_update=list(si.on_update))
                new.append(inst)
            if changed:
                bb.instructions = new


def build_mhsa():
    """Build the SPMD Bass program (identical on all cores; per-core inputs
    carry each core's batch slice and weight slices)."""
    from contextlib import ExitStack

    import bass_rust
    import concourse.bass as bass
    import concourse.mybir as mybir
    import concourse.tile as tile
    import concourse.vector_clock as vector_clock

    TC = _split_drain_tile_context(tile, bass_rust, vector_clock)
    f32 = mybir.dt.float32
    bf16 = mybir.dt.bfloat16
    Act = bass_rust.ActivationFunctionType

    nc = bass.Bass("TRN2", target_bir_lowering=False, debug=False,
                   num_devices=NCORES)
    xT = nc.dram_tensor("xT", [D, N], bf16, kind="ExternalInput").ap()
    wq = nc.dram_tensor("wq", [D, DS], bf16, kind="ExternalInput").ap()
    wk = nc.dram_tensor("wk", [D, DS], bf16, kind="ExternalInput").ap()
    wv = nc.dram_tensor("wv", [D, DS], bf16, kind="ExternalInput").ap()
    woT = nc.dram_tensor("woT", [DS, D], bf16, kind="ExternalInput").ap()
    ropec = nc.dram_tensor("ropec", [P, N], bf16, kind="ExternalInput").ap()
    ropes = nc.dram_tensor("ropes", [P, N], bf16, kind="ExternalInput").ap()
    tri = nc.dram_tensor("tri", [P, KC], bf16, kind="ExternalInput").ap()
    yT = nc.dram_tensor("yT", [D, N], bf16, kind="ExternalOutput").ap()

    with TC(nc) as tc, ExitStack() as ctx:
        pool = lambda name, bufs, **kw: ctx.enter_context(
            tc.tile_pool(name=name, bufs=bufs, **kw))

        consts = pool("consts", 1)
        # DMA order tracks first use: wq + the t=0 x tiles feed the first
        # matmuls, then wk/wv, then tables (Act DGE queue), woT last
        w_sb = {}
        for nm in ("wq", "wk", "wv"):
            w_sb[nm] = consts.tile([P, D // P, DS], bf16, tag=f"w_{nm}",
                                   name=f"w_{nm}")
        wq_r = wq.rearrange("(c p) m -> p c m", p=P)
        nc.sync.dma_start(out=w_sb["wq"][:, 0:4, :], in_=wq_r[:, 0:4, :])
        nc.scalar.dma_start(out=w_sb["wq"][:, 4:8, :], in_=wq_r[:, 4:8, :])
        ct_sb = consts.tile([P, N], bf16)
        st_sb = consts.tile([P, N], bf16)
        tri_sb = consts.tile([P, KC], bf16)
        woT_sb = consts.tile([P, DS // P, D], bf16)

        persist = pool("persist", 1)
        qT = persist.tile([P, 4, N], bf16, tag="qT")
        kT = persist.tile([P, 4, N], bf16, tag="kT")
        OT = persist.tile([P, 4, N], bf16, tag="OT")
        vsb = persist.tile([P, NKC, HPC * VW], bf16, tag="vsb")

        xt_pool = pool("xt", 16)
        cp_pool = pool("cp", 4)
        rp_pool = pool("rp", 6)
        es_pool = pool("es", 14)
        dn_pool = pool("dn", 2)
        oc_pool = pool("oc", 3)

        pp_pool = pool("pp", 1, space="PSUM")  # tags ps(3x2) + po(2x1) banks

        def ps_tile():
            return pp_pool.tile([P, 2, QT], f32, tag="ps", bufs=3, name="ps")

        # ones columns in V (softmax denominators), set once
        ones = bass.AP(tensor=vsb.tensor, offset=vsb.offset + 64,
                       ap=[vsb.ap[0], [HPC * VW, NKC], [VW, HPC], [1, 64]])
        nc.vector.memset(ones, 1.0)

        xts = {}

        def emit_xts_dma(t):
            tiles = []
            for c in range(D // P):
                xt = xt_pool.tile([P, QT], bf16, tag="xt", name=f"xt{t}_{c}")
                eng = nc.sync if c < 4 else nc.scalar
                eng.dma_start(
                    out=xt, in_=xT[c * P:(c + 1) * P, t * QT:(t + 1) * QT])
                tiles.append(xt)
            xts[t] = tiles

        def emit_qk_half(t, which, dc, pq, i):
            """8 accumulating matmuls: projection of dim-chunk dc into
            pq[:, i, :]."""
            for c in range(D // P):
                nc.tensor.matmul(pq[:, i, :],
                                 w_sb[which][:, c, dc * P:(dc + 1) * P],
                                 xts[t][c],
                                 start=(c == 0), stop=(c == D // P - 1))

        def emit_qk_rope(t, which, pair, pq):
            """Evict the psum pair through RoPE into qT/kT. The prologue
            (t=0) has an idle Act engine, so it takes the psum eviction
            there; later tiles keep it on DVE (Act is exp-bound then)."""
            tcols = slice(t * QT, (t + 1) * QT)
            cp = cp_pool.tile([P, 2, QT], bf16, tag="cp", name="cp")
            if t == 0:
                nc.scalar.copy(cp, pq)
            else:
                nc.vector.tensor_copy(cp, pq)
            dst = qT if which == "wq" else kT
            for i in (0, 1):
                dc = pair * 2 + i
                # DVE requires equal base partitions for two SBUF inputs, so
                # the sin table is stored block-swapped on the host: st[src]
                # holds the coefficient for dst = src +- 32. Output base may
                # differ (32-part ops route cross-quadrant via bank 0).
                swp = rp_pool.tile([P, QT], bf16, tag="swp", name="swp")
                for g in (0, 1):
                    o = g * 64
                    nc.vector.tensor_mul(swp[o:o + 32, :],
                                         st_sb[o + 32:o + 64, tcols],
                                         cp[o + 32:o + 64, i, :])
                    nc.vector.tensor_mul(swp[o + 32:o + 64, :],
                                         st_sb[o:o + 32, tcols],
                                         cp[o:o + 32, i, :])
                csn = rp_pool.tile([P, QT], bf16, tag="csn", name="csn")
                nc.vector.tensor_mul(csn, ct_sb[:, tcols], cp[:, i, :])
                nc.vector.tensor_add(dst[:, dc, tcols], csn, swp)

        def emit_v_half(t, tk, pv, i):
            for c in range(D // P):
                nc.tensor.matmul(pv[:, i, :],
                                 xts[t][c][:, tk * P:(tk + 1) * P],
                                 w_sb["wv"][:, c, :],
                                 start=(c == 0), stop=(c == D // P - 1))

        def emit_v_evict(t, pair, pv):
            ci0 = t * 4 + pair * 2
            src = bass.AP(tensor=pv.tensor, offset=pv.offset,
                          ap=[pv.ap[0], [pv.ap[1][0], 2], [64, HPC], [1, 64]])
            dstv = bass.AP(tensor=vsb.tensor,
                           offset=vsb.offset + ci0 * (HPC * VW),
                           ap=[vsb.ap[0], [HPC * VW, 2], [VW, HPC], [1, 64]])
            if t == 0:
                nc.scalar.copy(dstv, src)
            else:
                nc.vector.tensor_copy(dstv, src)

        # ---- filler units: always-ready PE work woven into attention ----
        filler = deque()

        def make_a_units(t):
            units = [lambda t=t: emit_xts_dma(t)]
            for pair in (0, 1):
                for which in ("wq", "wk"):
                    state = {}

                    def u1(t=t, which=which, pair=pair, state=state):
                        pq = ps_tile()
                        state["pq"] = pq
                        emit_qk_half(t, which, pair * 2, pq, 0)

                    def u2(t=t, which=which, pair=pair, state=state):
                        pq = state["pq"]
                        emit_qk_half(t, which, pair * 2 + 1, pq, 1)
                        emit_qk_rope(t, which, pair, pq)

                    units += [u1, u2]
            for pair in (0, 1):
                state = {}

                def v1(t=t, pair=pair, state=state):
                    pv = ps_tile()
                    state["pv"] = pv
                    emit_v_half(t, pair * 2, pv, 0)

                def v2(t=t, pair=pair, state=state):
                    pv = state["pv"]
                    emit_v_half(t, pair * 2 + 1, pv, 1)
                    emit_v_evict(t, pair, pv)

                units += [v1, v2]
            return units

        def make_c_units(qt):
            qcols = slice(qt * QT, (qt + 1) * QT)
            units = []
            for j in range(4):
                def cu(qt=qt, j=j, qcols=qcols):
                    pc = ps_tile()
                    for i in (0, 1):
                        m8 = j * 2 + i
                        for dcc in range(DS // P):
                            nc.tensor.matmul(
                                pc[:, i, :],
                                woT_sb[:, dcc, m8 * P:(m8 + 1) * P],
                                OT[:, dcc, qcols],
                                start=(dcc == 0), stop=(dcc == DS // P - 1))
                    oc = oc_pool.tile([P, 2, QT], bf16, tag="oc", name="oc")
                    nc.vector.tensor_copy(oc, pc)
                    for i in (0, 1):
                        m8 = j * 2 + i
                        nc.sync.dma_start(
                            out=yT[m8 * P:(m8 + 1) * P, qcols],
                            in_=oc[:, i, :])
                units.append(cu)
            return units

        def pop_filler(k=1):
            for _ in range(k):
                if filler:
                    filler.popleft()()

        # ---- attention ----
        def emit_scores(qt, h):
            """S + mask + exp stream for one head; returns the AV plan."""
            p0 = (h % 2) * 64
            dc = h // 2
            nkq = 4 * qt + 4
            av = []
            # diagonal chunks first (their masks/exp clear DVE/Act early),
            # merged in pairs: one exp covers both slices from the lower
            # chunk's column offset. The earlier columns of the upper slice
            # exp stale psum (finite scores/projections), and AV never reads
            # them.
            for mp in range(2):
                ps = ps_tile()
                base = mp * 2 * KC
                e = es_pool.tile([P, 2, QT], bf16, tag="es", name="es")
                for i in (0, 1):
                    m = mp * 2 + i
                    kc = 4 * qt + m
                    lo = m * KC
                    nc.tensor.matmul(
                        ps[:, i, lo:QT],
                        kT[p0:p0 + DK, dc, kc * KC:(kc + 1) * KC],
                        qT[p0:p0 + DK, dc, qt * QT + lo:(qt + 1) * QT],
                        start=True, stop=True)
                    av.append((kc, e, i, lo))
                nc.scalar.activation(e[:, :, base:QT], ps[:, :, base:QT],
                                     Act.Exp)
                # multiplicative causal mask on the boundary blocks: zeroes
                # masked entries in e before the denominator-summing AV (all
                # bf16 SBUF, 2x DVE; also keeps exp off the DVE dep chain)
                for i in (0, 1):
                    lo = (mp * 2 + i) * KC
                    nc.vector.tensor_mul(e[:, i, lo:lo + KC],
                                         e[:, i, lo:lo + KC], tri_sb)
            # off-diagonal pairs
            for pr in range(2 * qt):
                ps = ps_tile()
                for i in (0, 1):
                    kc = pr * 2 + i
                    nc.tensor.matmul(
                        ps[:, i, :],
                        kT[p0:p0 + DK, dc, kc * KC:(kc + 1) * KC],
                        qT[p0:p0 + DK, dc, qt * QT:(qt + 1) * QT],
                        start=True, stop=True)
                e = es_pool.tile([P, 2, QT], bf16, tag="es", name="es")
                nc.scalar.activation(e, ps, Act.Exp)
                av.append((pr * 2, e, 0, 0))
                av.append((pr * 2 + 1, e, 1, 0))
                if pr % 2 == 1:
                    pop_filler()
            av.sort(key=lambda z: z[0])
            return (qt, h, nkq, av)

        def emit_av(plan):
            """AV burst + denominator for one head."""
            qt, h, nkq, av = plan
            p0 = (h % 2) * 64
            dc = h // 2
            qcols = slice(qt * QT, (qt + 1) * QT)
            po = pp_pool.tile([P, QT], f32, tag="po", bufs=2, name="po")
            for kc, e, i, lo in av:
                nc.tensor.matmul(
                    po[:, lo:QT],
                    vsb[:, kc, h * VW:(h + 1) * VW],
                    e[:, i, lo:QT],
                    start=(kc == 0), stop=(kc == nkq - 1))
            # 1/d = exp(-ln d): Ln and Exp are co-resident in one Act
            # table set, so no table thrash; DVE reciprocal is the iterative
            # divide (~3.4us per 512 cols) and custom-DVE ops fail codegen.
            lnt = dn_pool.tile([64, QT], f32, tag="lnt", name="lnt")
            nc.scalar.activation(lnt, po[64:128, :], Act.Ln)
            rc = dn_pool.tile([64, QT], f32, tag="rc", name="rc")
            nc.scalar.activation(rc, lnt, Act.Exp, scale=-1.0)
            nc.vector.tensor_mul(OT[p0:p0 + DK, dc, qcols], po[0:64, :], rc)

        # ---- schedule ----
        # A(0) fully inline, Q/K before V: the V matmuls keep the PE busy
        # while DVE finishes the t=0 rope (AV(0,h0) needs all of V(0), so V
        # cannot trail into the filler queue - FIFO head-block). C(qt)
        # fillers are deferred to late q-tiles, which are exp-bound and need
        # the most always-ready PE work.
        emit_xts_dma(0)
        nc.sync.dma_start(out=w_sb["wk"],
                          in_=wk.rearrange("(c p) m -> p c m", p=P))
        nc.sync.dma_start(out=w_sb["wv"],
                          in_=wv.rearrange("(c p) m -> p c m", p=P))
        nc.scalar.dma_start(out=ct_sb, in_=ropec)
        nc.scalar.dma_start(out=st_sb, in_=ropes)
        nc.scalar.dma_start(out=tri_sb, in_=tri)
        nc.scalar.dma_start(out=woT_sb, in_=woT.rearrange("(c p) m -> p c m", p=P))
        for u in make_a_units(0)[1:]:
            u()
        a1_units = make_a_units(1)
        a1_units[0]()   # xts(1) DMA
        a1_units[1]()   # first Q half-group (8 MMs, no rope): PE work while
                        # the t=0 rope (DVE) finishes before B(0) scores

        pending = None
        for qt in range(NQT):
            if pending is not None:
                # flush before C fillers enter the queue: their matmuls
                # transitively depend on this AV burst (FIFO head-block)
                emit_av(pending)
                pending = None
            if qt == 0:
                filler.extend(a1_units[2:])
            elif qt + 1 < NQT:
                filler.extend(make_a_units(qt + 1))
            if qt == 3:
                for cj in range(3):
                    filler.extend(make_c_units(cj))
            for h in range(HPC):
                plan = emit_scores(qt, h)
                if pending is not None:
                    emit_av(pending)
                pending = plan
                pop_filler()
            # drain leftover fillers between q-tiles
            pop_filler(len(filler))
        emit_av(pending)
        for u in make_c_units(NQT - 1):
            u()
    _split_excess_waits(nc, mybir)
    return nc


def _rope_perm():
    """Per-head output-dim permutation: [evens, odds] per 64-dim head."""
    perm = []
    for h in range(HPC):
        perm += [h * DK + 2 * i for i in range(DK // 2)]
        perm += [h * DK + 2 * i + 1 for i in range(DK // 2)]
    return np.asarray(perm)


def host_inputs(x, Wq, Wk, Wv, Wo):
    """Shard + lay out inputs for each core. Returns list of in_maps."""
    x = np.asarray(x, np.float32)

    perm = _rope_perm()  # over 512 dims (8 heads)
    # RoPE tables in the permuted [evens, odds] partition layout; the
    # 128-partition tile covers 2 heads (identical pattern per 64 block).
    j = np.arange(DK // 2, dtype=np.float32)
    freqs = 1.0 / THETA ** (2.0 * j / DK)               # (32,)
    pos = np.arange(N, dtype=np.float32)
    ang = pos[None, :] * freqs[:, None]                 # (32, n)
    cos_t, sin_t = np.cos(ang), np.sin(ang)
    ct = np.empty((P, N), np.float32)
    st = np.empty((P, N), np.float32)
    # st is stored block-swapped: row p holds the coefficient applied while
    # READING partition p, whose product lands at partition p -+ 32 (see
    # emit_qk_rope). Evens-slot output (p-32) needs -sin; odds-slot (p+32)
    # needs +sin.
    for g in range(2):
        o = g * DK
        ct[o:o + 32] = cos_t
        ct[o + 32:o + 64] = cos_t
        st[o:o + 32] = sin_t
        st[o + 32:o + 64] = -sin_t

    # Triangular multiplicative 0/1 mask for a diagonal boundary block.
    i = np.arange(KC)[:, None]
    jj = np.arange(KC)[None, :]
    tri = np.where(jj >= i, 1.0, 0.0).astype(BF16NP)

    bf = lambda a: np.ascontiguousarray(a).astype(BF16NP)
    scale = 1.0 / np.sqrt(np.float32(DK))
    in_maps = []
    for c in range(NCORES):
        b, a = divmod(c, 2)
        sl = slice(a * DS, (a + 1) * DS)
        xTb = x[b].reshape(N, D).T                       # (D, N)
        wq_c = (np.asarray(Wq, np.float32)[sl, :][perm, :] * scale).T
        wk_c = np.asarray(Wk, np.float32)[sl, :][perm, :].T
        wv_c = np.asarray(Wv, np.float32)[sl, :].T
        woT_c = np.asarray(Wo, np.float32)[:, sl].T
        in_maps.append({
            "xT": bf(xTb),
            "wq": bf(wq_c),
            "wk": bf(wk_c),
            "wv": bf(wv_c),
            "woT": bf(woT_c),
            "ropec": bf(ct),
            "ropes": bf(st),
            "tri": tri,
        })
    return in_maps


def host_gather(results):
    """Sum per-core partial yT outputs and restore (b, n, D) layout."""
    y = np.empty((B, N, D), np.float32)
    for b in range(B):
        acc = results[2 * b]["yT"].astype(np.float32) \
            + results[2 * b + 1]["yT"].astype(np.float32)
        y[b] = acc.T
    return y


def kernel(x, Wq, Wk, Wv, Wo):
    from concourse.bass_utils import run_bass_kernel_spmd

    nc = build_mhsa()
    in_maps = host_inputs(x, Wq, Wk, Wv, Wo)
    res = run_bass_kernel_spmd(nc, in_maps, list(range(NCORES)))
    return host_gather(res.results)


if __name__ == "__main__":
    rng = np.random.default_rng(0)
    x = rng.standard_normal((B, N, D), dtype=np.float32)
    std = (2.0 / (D + D)) ** 0.5
    ws = [rng.standard_normal((D, D), dtype=np.float32) * std for _ in range(4)]
    y = kernel(x, *ws)
    print("kernel ran, output", y.shape, y.dtype)
